# revision 13
# baseline (speedup 1.0000x reference)
"""Contrastive (NT-Xent) loss kernel for 8 Trainium2 NeuronCores.

Math (reference): z = l2norm(concat(proj_1, proj_2)) [8192,128];
sim = z @ z.T; loss = mean_i( log(sum_{j!=i} exp(2*sim_ij)) - 2*pos_i ).

Symmetric scheme: sim is symmetric, so each unordered pair {i,j} is
computed ONCE. Work partition (rotation-invariant, SPMD-identical):
core c (data rotated by c*1024 rows, local view) computes, for its 1024
rows (local block 0):
  - diag block (0,0): strict LOWER triangle (col < row)
  - blocks 1..3 (cols [1024,4096)): full rectangles
  - block 4 (cols [4096,5120)): strict UPPER triangle (col-4096 > row)
  - distance-4096 pairs (the positives) via an elementwise dot; their
    exp(2*pos) enters both partners' denominators analytically.
Per m-tile (128 rows) that is a constant 4224 columns -> 52% of the
baseline exp work, perfectly balanced.

Each exp'd tile contributes row sums (ACT accum) AND column sums
(ones-matmul on PE into a persistent PSUM accumulator). Column sums of
blocks 1..4 belong to cores c+1..c+4: one AllGather moves all 8x4
vectors; a host-supplied 0/1 weight vector (per-core, undoing the
rotation) + one PE matmul sums the 4 incoming vectors on each core.
Host sums the 8 partial scalar losses (as before).

PSUM budget (8 banks): 2 x [128,1536] band slots (3 banks each) +
[8,512] send col-acc + [8,512] local col-acc.
"""

import ml_dtypes
import numpy as np

import concourse.bass as bass
import concourse.tile as tile
from concourse import bacc, mybir
from concourse.bass_utils import run_bass_kernel_spmd
from concourse.hw_specs import get_activation_tables
from concourse.masks import make_identity

B = 4096
D = 128
N2 = 2 * B            # 8192 total rows
NCORES = 8
RPC = N2 // NCORES    # 1024 rows per core
COLS = 5 * RPC        # 5120 local columns needed
NCH = COLS // 512     # 10 input chunks of 512 rows
TEMP = 0.5
MASKV = -30.0         # additive pre-exp mask: exp(2*(x-30)) ~ 0

F32 = mybir.dt.float32
BF16 = mybir.dt.bfloat16
AX = mybir.AxisListType
OP = mybir.AluOpType
AF = mybir.ActivationFunctionType

LAST_RESULT = None  # BassKernelResults of the most recent run (for test.py)


def _build_nc():
    nc = bacc.Bacc("TRN2", target_bir_lowering=False)
    xn_d = nc.declare_dram_parameter("xn", [COLS, D], BF16, isOutput=False)
    ws_d = nc.declare_dram_parameter("wsel", [64, 16], F32, isOutput=False)
    out_d = nc.declare_dram_parameter("out", [1, 1], F32, isOutput=True)

    inb = nc.dram_tensor("inb", [8, 512], F32)          # 4 outgoing vectors
    outb = nc.dram_tensor("outb", [64, 512], F32)       # gathered 8x4x2
    scr_d = nc.dram_tensor("scr", [6, 512], F32)        # local vectors bounce

    table_names = list(get_activation_tables(nc.m.arch).keys())
    combined_id = table_names.index("natural_log_exp_and_others")

    with tile.TileContext(nc) as tc:
        with (
            tc.tile_pool(name="big", bufs=1) as big,
            tc.tile_pool(name="work", bufs=3) as work,
            tc.tile_pool(name="ps", bufs=2, space="PSUM") as ps,
            tc.tile_pool(name="caccs", bufs=1, space="PSUM") as caccs,
            tc.tile_pool(name="caccl", bufs=1, space="PSUM") as caccl,
        ):
            nc.scalar.add_instruction(mybir.InstLoadActFuncSet(
                name=nc.get_next_instruction_name(), ins=[], outs=[],
                act_func_set_id=combined_id))

            xn_all = big.tile([128, NCH * 4, 128], BF16, tag="xn")
            xhat = big.tile([D, COLS], BF16, tag="xhat")
            ns_c = big.tile([128, NCH * 4], F32, tag="ns")
            lnn = big.tile([128, NCH * 4], F32, tag="lnn")
            s_c = big.tile([128, NCH * 4], F32, tag="s")
            ones_b = big.tile([128, 1], BF16, tag="ones_b")
            ones_f = big.tile([128, 1], F32, tag="ones_f")
            rs_all = big.tile([128, 32], F32, tag="rs")   # (m, j) row sums
            ident = big.tile([128, 128], BF16, tag="ident")
            mk_low = big.tile([128, 128], F32, tag="mklow")   # kill c>=r
            mk_up = big.tile([128, 128], F32, tag="mkup")     # kill c<=r
            wsel = big.tile([64, 16], F32, tag="wsel")
            oneh = big.tile([128, 64], BF16, tag="oneh")  # 8 one-hot lhsTs
            sb_g = big.tile([64, 512], F32, tag="sbg")    # gathered
            sv_send = big.tile([8, 512], F32, tag="svs")  # send evac
            sv_loc = big.tile([6, 512], F32, tag="svl")   # local evac

            # persistent column-sum accumulators (zeroed, then matmuls
            # accumulate with start=False)
            cs = caccs.tile([8, 512], F32, tag="cs")   # q2..q9 (blocks 1-4)
            cl = caccl.tile([8, 512], F32, tag="cl")   # q0,q1,pos0,pos1,in0,in1

            nc.vector.memset(ones_f, 1.0)
            nc.vector.memset(ones_b, 1.0)
            nc.vector.memset(oneh, 0.0)
            for r in range(8):
                nc.vector.memset(oneh[:, r * 8 + r:r * 8 + r + 1], 1.0)
            nc.vector.memset(cs, 0.0)
            nc.vector.memset(cl, 0.0)
            make_identity(nc, ident[:])
            # diag corner: keep strict lower (r > c): fill where NOT(r-c>0)
            nc.gpsimd.memset(mk_low, 0.0)
            nc.gpsimd.affine_select(
                out=mk_low, in_=mk_low, compare_op=OP.is_gt, fill=MASKV,
                base=0, pattern=[[-1, 128]], channel_multiplier=1)
            # b4 corner: keep strict upper (c > r): (c - r) > 0
            nc.gpsimd.memset(mk_up, 0.0)
            nc.gpsimd.affine_select(
                out=mk_up, in_=mk_up, compare_op=OP.is_gt, fill=MASKV,
                base=0, pattern=[[1, 128]], channel_multiplier=-1)
            nc.sync.dma_start(out=wsel, in_=ws_d[:, :])

            def prep_chunk(c):
                nc.sync.dma_start(
                    out=xn_all[:, c * 4:(c + 1) * 4, :],
                    in_=xn_d[c * 512:(c + 1) * 512, :].rearrange(
                        "(t p) d -> p t d", p=128),
                )
                for j in range(4):
                    jj = c * 4 + j
                    sqs = work.tile([128, 128], F32, tag="sqs")
                    blk = xn_all[:, jj, :]
                    nc.vector.scalar_tensor_tensor(
                        out=sqs, in0=blk, scalar=1.0, in1=blk,
                        op0=OP.mult, op1=OP.mult,
                        accum_out=ns_c[:, jj:jj + 1],
                    )
                gsl = slice(c * 4, c * 4 + 4)
                with tc.high_priority():
                    nc.scalar.activation(
                        out=lnn[:, gsl], in_=ns_c[:, gsl], func=AF.Ln)
                    nc.scalar.activation(
                        out=s_c[:, gsl], in_=lnn[:, gsl], func=AF.Exp,
                        scale=-0.5)
                xsc = work.tile([128, 4, 128], BF16, tag="xsc")
                nc.vector.tensor_mul(
                    xsc, xn_all[:, gsl, :],
                    s_c[:, gsl].broadcast_to([128, 4, 128]))
                tp = ps.tile([128, 512], BF16, tag="ps")
                for j in range(4):
                    nc.tensor.transpose(
                        tp[:, j * 128:(j + 1) * 128], xsc[:, j, :], ident[:])
                nc.vector.tensor_copy(xhat[:, c * 512:(c + 1) * 512], tp[:])

            def band_exp(m, pieces, slot_idx, corners=()):
                """One exp over a PSUM slot packed with `pieces`
                [(xhat_col, ncols), ...]; corners: (slot_off, mask) to add
                pre-exp. Row sums -> rs_all[:, m*4+slot_idx]; returns the
                sc tile + the piece layout for colsum matmuls."""
                tot = sum(n for _, n in pieces)
                pst = ps.tile([128, 1536], F32, tag="ps")
                lhsT = xhat[:, m * 128:(m + 1) * 128]
                off = 0
                for col, ncol in pieces:
                    nc.tensor.matmul(
                        pst[:, off:off + ncol], lhsT=lhsT,
                        rhs=xhat[:, col:col + ncol],
                        start=True, stop=True)
                    off += ncol
                for c_off, mk in corners:
                    nc.vector.tensor_add(
                        pst[:, c_off:c_off + 128],
                        pst[:, c_off:c_off + 128], mk)
                sc = work.tile([128, 1536], BF16, tag="sc")
                nc.scalar.activation(
                    out=sc[:, 0:tot], in_=pst[:, 0:tot], func=AF.Exp,
                    scale=1.0 / TEMP,
                    accum_out=rs_all[:, m * 4 + slot_idx:m * 4 + slot_idx + 1],
                )
                return sc

            def colsum(sc, sc_off, n, acc, row, acc_off):
                """acc[row, acc_off:+n] += column sums of sc[:, sc_off:+n].
                lhsT is a one-hot column matrix so the out spans partitions
                0..8 (matmul requires base partition 0); other rows += 0."""
                nc.tensor.matmul(
                    acc[:, acc_off:acc_off + n],
                    lhsT=oneh[:, row * 8:row * 8 + 8],
                    rhs=sc[:, sc_off:sc_off + n],
                    start=False, stop=True, skip_group_check=True)

            # ---------------- emission ----------------
            for c in range(5):
                prep_chunk(c)

            # LOOP A: blocks b1 + b2-first-half (cols 1024..2560)
            for m in range(8):
                sc = band_exp(m, [(1024, 512), (1536, 512), (2048, 512)], 0)
                for i in range(3):
                    colsum(sc, i * 512, 512, cs, i, 0)     # q2,q3,q4

            for c in range(5, 8):
                prep_chunk(c)

            # LOOP B: b2-second-half + b3 (cols 2560..4096)
            for m in range(8):
                sc = band_exp(m, [(2560, 512), (3072, 512), (3584, 512)], 1)
                for i in range(3):
                    colsum(sc, i * 512, 512, cs, 3 + i, 0)  # q5,q6,q7

            for c in range(8, 10):
                prep_chunk(c)

            # positives: pos_r = xhat[:,r] . xhat[:,4096+r]
            prod = big.tile([128, RPC], BF16, tag="prod")
            nc.vector.tensor_mul(prod, xhat[:, 0:RPC], xhat[:, B:B + RPC])
            colsum(prod, 0, 512, cl, 2, 0)
            colsum(prod, 512, 512, cl, 3, 0)

            # LOOP C: block4 strict-upper triangle, cols [4096+128m, 5120)
            for m in range(8):
                L = 1024 - 128 * m
                pieces = [(4096 + 128 * m, 128)]
                rem = L - 128
                col = 4096 + 128 * m + 128
                while rem > 0:
                    n = min(512, rem)
                    pieces.append((col, n))
                    col += n
                    rem -= n
                sc = band_exp(m, pieces, 2, corners=((0, mk_up),))
                # colsums -> q8 (cols 4096..4608), q9 (4608..5120)
                if m <= 3:
                    colsum(sc, 0, 512 - 128 * m, cs, 6, 128 * m)
                    colsum(sc, 512 - 128 * m, 512, cs, 7, 0)
                else:
                    colsum(sc, 0, L, cs, 7, 128 * m - 512)

            # send: cs -> SBUF -> inb, AllGather, readback
            nc.vector.tensor_copy(sv_send, cs[:])
            nc.sync.dma_start(out=inb[:, :], in_=sv_send[:])
            nc.gpsimd.collective_compute(
                "AllGather", OP.bypass,
                ins=[inb.ap().opt()], outs=[outb.ap().opt()],
                replica_groups=[list(range(NCORES))],
            )
            nc.sync.dma_start(out=sb_g, in_=outb[:, :])

            # LOOP D: diag strict-lower triangle, cols [0, 128(m+1))
            for m in range(8):
                L = 128 * (m + 1)
                pieces = []
                col = 0
                rem = 128 * m
                while rem > 0:
                    n = min(512, rem)
                    pieces.append((col, n))
                    col += n
                    rem -= n
                pieces.append((128 * m, 128))  # corner
                sc = band_exp(m, pieces, 3, corners=((128 * m, mk_low),))
                if L <= 512:
                    colsum(sc, 0, L, cl, 0, 0)
                else:
                    colsum(sc, 0, 512, cl, 0, 0)
                    colsum(sc, 512, L - 512, cl, 1, 0)

            # combine incoming: row 4 (resp 5) of cl += w^T @ gathered half;
            # wsel[:, 0:8] has w in column 4, wsel[:, 8:16] in column 5.
            nc.tensor.matmul(cl[:, :], lhsT=wsel[:, 0:8], rhs=sb_g[:, :],
                             start=False, stop=True, skip_group_check=True)
            nc.tensor.matmul(cl[:, :], lhsT=wsel[:, 8:16], rhs=sb_g[:, :],
                             start=False, stop=True, skip_group_check=True)

            # evacuate local vectors, bounce via DRAM, repack per-row [128,8]
            nc.vector.tensor_copy(sv_loc, cl[0:6, :])
            nc.sync.dma_start(out=scr_d[:, :], in_=sv_loc[:])
            ownc_t = big.tile([128, 8], F32, tag="ownc")
            pos_t = big.tile([128, 8], F32, tag="post")
            inc_t = big.tile([128, 8], F32, tag="inct")
            for t_tile, r0 in ((ownc_t, 0), (pos_t, 2), (inc_t, 4)):
                nc.sync.dma_start(
                    out=t_tile,
                    in_=scr_d[r0:r0 + 2, :].rearrange(
                        "q (m2 p) -> p (q m2)", p=128))

            # den = rowsums + own colsums + incoming + exp(2*pos)
            rowsum = big.tile([128, 8], F32, tag="rowsum")
            nc.vector.tensor_reduce(
                out=rowsum, in_=rs_all[:].rearrange("p (m j) -> p m j", j=4),
                axis=AX.X, op=OP.add)
            epos = big.tile([128, 8], F32, tag="epos")
            nc.scalar.activation(out=epos, in_=pos_t, func=AF.Exp,
                                 scale=1.0 / TEMP)
            den = big.tile([128, 8], F32, tag="den")
            nc.vector.tensor_add(den, rowsum, ownc_t)
            nc.vector.tensor_add(den, den, inc_t)
            nc.vector.tensor_add(den, den, epos)
            logden = big.tile([128, 8], F32, tag="logden")
            nc.scalar.activation(out=logden, in_=den, func=AF.Ln)
            # loss_row = logden - 2*pos
            lrow = big.tile([128, 8], F32, tag="lrow")
            nc.vector.scalar_tensor_tensor(
                out=lrow, in0=pos_t, scalar=-1.0 / TEMP, in1=logden,
                op0=OP.mult, op1=OP.add)
            lps = ps.tile([1, 8], F32, tag="ps")
            nc.tensor.matmul(lps, lhsT=ones_f, rhs=lrow, start=True, stop=True)
            l1 = big.tile([1, 1], F32, tag="l1")
            nc.vector.tensor_reduce(out=l1, in_=lps, axis=AX.X, op=OP.add)
            res = big.tile([1, 1], F32, tag="res")
            nc.vector.tensor_scalar_mul(out=res, in0=l1, scalar1=1.0 / N2)
            nc.sync.dma_start(out=out_d[:, :], in_=res)

    nc.compile()
    return nc


_NC = None


def kernel(proj_1: np.ndarray, proj_2: np.ndarray) -> np.ndarray:
    global _NC, LAST_RESULT
    import os

    reps = np.concatenate(
        [np.asarray(proj_1, np.float32), np.asarray(proj_2, np.float32)], axis=0
    )
    assert reps.shape == (N2, D)

    in_maps = []
    for c in range(NCORES):
        xn = np.roll(reps, -c * RPC, axis=0)[:COLS]
        w = np.zeros((64, 16), np.float32)
        for k in (1, 2, 3, 4):
            idx = 4 * ((c - k) % NCORES) + (k - 1)
            w[2 * idx, 4] = 1.0          # first halves -> cl row 4
            w[2 * idx + 1, 8 + 5] = 1.0  # second halves -> cl row 5
        in_maps.append({
            "xn": np.ascontiguousarray(xn).astype(ml_dtypes.bfloat16),
            "wsel": w,
        })

    if _NC is None:
        _NC = _build_nc()

    trace = bool(os.environ.get("CONTRASTIVE_TRACE"))
    result = run_bass_kernel_spmd(
        _NC, in_maps, core_ids=list(range(NCORES)), trace=trace
    )
    LAST_RESULT = result
    total = sum(float(r["out"][0, 0]) for r in result.results)
    return np.float32(total)
